# revision 37
# baseline (speedup 1.0000x reference)

# Trainium2 Bass kernel for MinConvExpLSTMCell.
#
# Math (linear-space reformulation of the reference's log-space scan):
#   y = conv3x3(x, W) + b; [f_gate, i_gate, h_tilde] = split(y)
#   diff = f_gate - i_gate = conv(x, W_f - W_i) + (b_f - b_i)
#   f = sigmoid(diff);  i = 1 - f
#   g = max(sigmoid(ht), ht + 0.5)              (exact identity for g(ht))
#   h_t = f_t * h_{t-1} + i_t * g_t,  h_{-1} = g(h0)
#
# Sharding: 8 cores = 4 batches x 2 spatial halves (16 output rows each,
# 1 halo row; images stored 18 rows x 32 cols, vertical pad only).
#
# Conv: fp8 DoubleRow matmuls (0.5 cyc/row), 8 per timestep. Horizontal
# taps come from pre-shifted column-variant copies of the image so every
# window is a CONTIGUOUS 512-elem span (HW DR needs [K, 2, N] APs).
# Partition groups per region:
#   T1 = [x_hi(c0); x_hi(c-1)]   T2 = [x_hi(c+1); x_lo16(c+1)]
#   T3 = [x_lo16(c0); x_lo16(c-1)]   ONES (bias rows, Dekker fp8 pair)
# x_hi = fp8(x), x_lo16 = fp8(16(x - x_hi)). Weights are 2-term fp8
# Dekker pairs (w_hi + w_lo planes); diff out-channels on a x64 grid
# (fp8 subnormal avoidance), rescaled inside the sigmoid via a
# per-partition scale AP; ht at x1 (consumed raw).
#
# PSUM [64*(diff+bd); ht+bh+0.5] consumers: one full-width sigmoid on
# the scalar engine -> f (top), s (bottom); one Copy drain of the ht
# half -> YH. DVE: pixel-split i = 1-f (4x mode), F2 = 1-I2, fused
# split g = max(s, YH), and the per-pixel tensor_tensor_scan. gpsimd:
# u = i*g (in place on I2), scan-reset memset, h chain copy. All gates
# fp16. Scan layout pixel-major, time-minor, 9 slots/pixel with an f=0
# reset column chaining segments.

import sys
import numpy as np

sys.path.insert(0, "/opt/trn_rl_repo")

import ml_dtypes
from contextlib import ExitStack

import concourse.bass as bass
import bass_rust
import concourse.bacc as bacc
import concourse.mybir as mybir
from concourse.tile import TileContext
from concourse.bass_utils import run_bass_kernel_spmd

F8 = ml_dtypes.float8_e4m3
F16 = np.float16
B, T, C, H, W = 4, 64, 64, 32, 32
SEG = 8
NSEG = T // SEG
HP, WD32 = 18, 32          # padded rows x cols (no horizontal pad)
RC = HP * WD32             # 576
NPX = 16 * 32              # 512 output pixels per core
HPX = NPX // 2             # 256 pixels per partition-half
TS = SEG + 1               # 9 scan slots per pixel per segment
NF2 = HPX * TS             # 2304 pixel-split scan free size
# per-segment stream: [T1 | T2 | T3] (SEG*RC each) + ones block (RC)
SEGR = SEG * RC
SEGX = 3 * SEGR + RC
OFF_T1, OFF_T2, OFF_T3, OFF_ONES = 0, SEGR, 2 * SEGR, 3 * SEGR

# DR plan: 8 matmuls/timestep, each = (planeA, planeB); plane =
# (region, r0, wkind). wkind: 'hi'/'lo' = Dekker weight terms,
# 'b' = bias ones plane.
DR_PLAN = [
    (("T1", 0, "hi"), ("T1", 0, "lo")),
    (("T1", 1, "hi"), ("T1", 1, "lo")),
    (("T1", 2, "hi"), ("T1", 2, "lo")),
    (("T2", 0, "hi"), ("T2", 0, "lo")),
    (("T2", 1, "hi"), ("T2", 1, "lo")),
    (("T2", 2, "hi"), ("T2", 2, "lo")),
    (("T3", 0, "hi"), ("T3", 1, "hi")),
    (("T3", 2, "hi"), ("ONES", 0, "b")),
]
NPL = 16
DIRECT_G = set()        # (direct-PSUM g reads hurt; keep empty)

_CACHE = {}


def _planes(win, d):
    """Insert a [stride=d, count=2] plane dim after the partition dim."""
    c = win.copy()
    dims = [list(x) for x in win.ap]
    c.ap = bass_rust.VecI64Pair([dims[0]] + [[d, 2]] + dims[1:])
    return c


def _off(region, ti, r0):
    if region == "ONES":
        return OFF_ONES
    base = {"T1": OFF_T1, "T2": OFF_T2, "T3": OFF_T3}[region]
    return base + ti * RC + r0 * WD32


def _build():
    f32 = mybir.dt.float32
    f16 = mybir.dt.float16
    f8 = mybir.dt.float8e4
    AF = mybir.ActivationFunctionType
    OP = mybir.AluOpType
    DRM = mybir.MatmulPerfMode.DoubleRow

    nc = bacc.Bacc()
    xs = nc.dram_tensor("xs", [128, NSEG * SEGX], f8, kind="ExternalInput")
    wt = nc.dram_tensor("wt", [128, NPL * 128], f8, kind="ExternalInput")
    cst = nc.dram_tensor("cst", [128, 2 + HPX], f32, kind="ExternalInput")
    out = nc.dram_tensor("out", [NSEG, 128, NF2], f16, kind="ExternalOutput")

    with TileContext(nc) as tc, ExitStack() as ctx:
        cpool = ctx.enter_context(tc.tile_pool(name="consts", bufs=1))
        xpool = ctx.enter_context(tc.tile_pool(name="x", bufs=4))
        pspool = ctx.enter_context(tc.tile_pool(name="ps", bufs=2, space="PSUM"))
        sdpool = ctx.enter_context(tc.tile_pool(name="sd", bufs=3))
        yhpool = ctx.enter_context(tc.tile_pool(name="yh", bufs=3))
        fpool = ctx.enter_context(tc.tile_pool(name="f", bufs=2))
        ipool = ctx.enter_context(tc.tile_pool(name="i", bufs=2))
        gpool = ctx.enter_context(tc.tile_pool(name="g", bufs=3))
        hpool = ctx.enter_context(tc.tile_pool(name="h", bufs=2))

        w_sb = cpool.tile([128, NPL * 128], f8)
        nc.gpsimd.dma_start(w_sb[:, :], wt[:, :])
        cst_sb = cpool.tile([128, 2 + HPX], f32)
        nc.gpsimd.dma_start(cst_sb[:, :], cst[:, :])
        scalep = cst_sb[:, 0:1]            # [1/64 ; 1]
        biasp = cst_sb[:, 1:2]             # [0 ; -0.5]
        g0 = cst_sb[:, 2:2 + HPX]          # g(h0), pixel-split

        wv = w_sb.rearrange("p (j two m) -> p j two m", two=2, m=128)

        # preload the sigmoid act table while input DMAs stream
        warm = cpool.tile([128, 1], f16)
        nc.scalar.activation(warm[:, :], cst_sb[:, 0:1], AF.Sigmoid)

        def emit_matmuls(ps, xt, hf):
            for j, (pa, pb) in enumerate(DR_PLAN):
                lhsT = wv[:, j]
                for k in range(4):
                    ti = hf * 4 + k
                    oa = _off(pa[0], ti, pa[1])
                    ob = _off(pb[0], ti, pb[1])
                    win = xt[:, oa:oa + 512]
                    nc.tensor.matmul(
                        ps[:, k * 512:(k + 1) * 512], lhsT,
                        _planes(win, ob - oa),
                        start=(j == 0), stop=(j == len(DR_PLAN) - 1),
                        perf_mode=DRM)

        def emit_half_gates(tl, hf, yh_direct=None):
            (SDv, YHv, F2, I2, G2, F2v, I2v, G2v) = tl
            lo, hi = 4 * hf, 4 * hf + 4
            slo, shi = lo + 1, hi + 1
            # g = max(s, yh), dense read -> pixel-split write (DVE tt).
            # yh_direct: read ht straight from PSUM (no Act drain for
            # this half; slower DVE op but relieves the scalar engine).
            # g goes first on the DVE queue so PSUM frees promptly.
            if yh_direct is not None:
                ya = yh_direct[64:128, 0:HPX, :]
                yb = yh_direct[64:128, HPX:NPX, :]
            else:
                ya = YHv[64:128, 0:HPX, lo:hi]
                yb = YHv[64:128, HPX:NPX, lo:hi]
            nc.vector.tensor_tensor(
                G2v[0:64, :, lo:hi], SDv[64:128, 0:HPX, lo:hi], ya, OP.max)
            nc.vector.tensor_tensor(
                G2v[64:128, :, lo:hi], SDv[64:128, HPX:NPX, lo:hi], yb,
                OP.max)
            # i = 1 - f, pixel-split (DVE ts 4x mode)
            nc.vector.tensor_scalar(
                I2v[0:64, :, slo:shi], SDv[0:64, 0:HPX, lo:hi], -1.0, 1.0,
                OP.mult, OP.add)
            nc.vector.tensor_scalar(
                I2v[64:128, :, slo:shi], SDv[0:64, HPX:NPX, lo:hi], -1.0, 1.0,
                OP.mult, OP.add)
            # f = 1 - i, full width from I2 (DVE ts 4x mode)
            nc.vector.tensor_scalar(
                F2v[:, :, slo:shi], I2v[:, :, slo:shi], -1.0, 1.0,
                OP.mult, OP.add)
            # u = i * g, in place on I2 (gpsimd tt mult)
            nc.gpsimd.tensor_tensor(
                I2v[:, :, slo:shi], I2v[:, :, slo:shi], G2v[:, :, lo:hi],
                OP.mult)

        def emit_scan(tl, s_idx):
            (SDv, YHv, F2, I2, G2, F2v, I2v, G2v) = tl
            H2 = hpool.tile([128, NF2], f16)
            nc.vector.tensor_tensor_scan(
                H2[:, :], F2[:, :], I2[:, :], 0.0, OP.mult, OP.add)
            nc.sync.dma_start(out[s_idx], H2[:, :])
            return H2

        def fetch_x(s):
            # T1+T2 on the SP queue, T3+ones on the gpsimd queue: the
            # sim charges DMA transfer time to the issuing queue, so
            # splitting the byte volume matters more than issue count
            xt = xpool.tile([128, SEGX], f8)
            o = s * SEGX
            nc.sync.dma_start(xt[:, 0:SEGR], xs[:, o:o + SEGR])
            nc.sync.dma_start(
                xt[:, SEGR:2 * SEGR], xs[:, o + SEGR:o + 2 * SEGR])
            nc.gpsimd.dma_start(
                xt[:, 2 * SEGR:SEGX], xs[:, o + 2 * SEGR:o + SEGX])
            return xt

        prev = None
        h_prev = None
        xt_next = fetch_x(0)

        for s in range(NSEG):
            xt = xt_next
            if s + 1 < NSEG:
                xt_next = fetch_x(s + 1)

            SD = sdpool.tile([128, NPX * SEG], f16)       # f top / s bottom
            SDv = SD.rearrange("p (px t) -> p px t", t=SEG)
            YH = yhpool.tile([128, NPX * SEG], f16)       # ht+bh+0.5 (bottom)
            YHv = YH.rearrange("p (px t) -> p px t", t=SEG)
            F2 = fpool.tile([128, NF2], f16)
            I2 = ipool.tile([128, NF2], f16)
            G2 = gpool.tile([128, HPX * SEG], f16)
            cur = (SDv, YHv, F2, I2, G2,
                   F2.rearrange("p (px t) -> p px t", t=TS),
                   I2.rearrange("p (px t) -> p px t", t=TS),
                   G2.rearrange("p (px t) -> p px t", t=SEG))

            # reset column for the per-pixel scan chains
            nc.gpsimd.memset(cur[5][:, :, 0], 0.0)

            for hf in range(2):
                ps = pspool.tile([128, 4 * 512], f32)
                psx = ps.rearrange("p (k x) -> p x k", k=4)
                lo, hi = 4 * hf, 4 * hf + 4
                emit_matmuls(ps, xt, hf)
                last_half = (s == NSEG - 1 and hf == 1)
                if not last_half:
                    # f|s = sigmoid(psum * [1/64;1] + [0;-0.5]), full width
                    nc.scalar.activation(
                        SDv[:, :, lo:hi], psx[:, :, :], AF.Sigmoid,
                        bias=biasp, scale=scalep)
                    # drain ht+bh+0.5 for g's linear branch
                    nc.scalar.activation(
                        YHv[64:128, :, lo:hi], psx[64:128, :, :], AF.Copy)
                if hf == 0:
                    # u col0 = h_{-1} (chains segments)
                    if prev is not None:
                        h_prev = emit_scan(prev, s - 1)
                        hp_px = h_prev.rearrange("p (px t) -> p px t", t=TS)
                        nc.gpsimd.tensor_scalar(
                            cur[6][:, :, 0], hp_px[:, :, SEG], 1.0, 0.0,
                            OP.mult, OP.add)
                    else:
                        nc.gpsimd.tensor_scalar(
                            cur[6][:, :, 0], g0, 1.0, 0.0, OP.mult, OP.add)
                if last_half:
                    # last half: pixel-quarter pipeline so the tail
                    # overlaps sigmoid/drain/gates/scan/DMA across engines
                    (SDv2, YHv2, F2_, I2_, G2_, F2v2, I2v2, G2v2) = cur
                    H2t = hpool.tile([128, NF2], f16)
                    edges = [0, 64, 128, 192, 256]
                    for q in range(4):
                        e0, e1 = edges[q], edges[q + 1]
                        A = slice(e0, e1)
                        A2 = slice(256 + e0, 256 + e1)
                        pq = slice(e0, e1)
                        for dr in (A, A2):
                            nc.scalar.activation(
                                SDv2[:, dr, lo:hi], psx[:, dr, :],
                                AF.Sigmoid, bias=biasp, scale=scalep)
                            nc.scalar.activation(
                                YHv2[64:128, dr, lo:hi], psx[64:128, dr, :],
                                AF.Copy)
                        nc.vector.tensor_scalar(
                            I2v2[0:64, pq, lo + 1:hi + 1],
                            SDv2[0:64, A, lo:hi], -1.0, 1.0, OP.mult, OP.add)
                        nc.vector.tensor_scalar(
                            I2v2[64:128, pq, lo + 1:hi + 1],
                            SDv2[0:64, A2, lo:hi], -1.0, 1.0, OP.mult, OP.add)
                        nc.vector.tensor_scalar(
                            F2v2[:, pq, lo + 1:hi + 1],
                            I2v2[:, pq, lo + 1:hi + 1], -1.0, 1.0,
                            OP.mult, OP.add)
                        nc.vector.tensor_tensor(
                            G2v2[0:64, pq, lo:hi], SDv2[64:128, A, lo:hi],
                            YHv2[64:128, A, lo:hi], OP.max)
                        nc.vector.tensor_tensor(
                            G2v2[64:128, pq, lo:hi], SDv2[64:128, A2, lo:hi],
                            YHv2[64:128, A2, lo:hi], OP.max)
                        nc.gpsimd.tensor_tensor(
                            I2v2[:, pq, lo + 1:hi + 1],
                            I2v2[:, pq, lo + 1:hi + 1],
                            G2v2[:, pq, lo:hi], OP.mult)
                        lo2, hi2 = e0 * TS, e1 * TS
                        nc.vector.tensor_tensor_scan(
                            H2t[:, lo2:hi2], F2_[:, lo2:hi2], I2_[:, lo2:hi2],
                            0.0, OP.mult, OP.add)
                        nc.sync.dma_start(
                            out[NSEG - 1, :, lo2:hi2], H2t[:, lo2:hi2])
                else:
                    emit_half_gates(
                        cur, hf,
                        yh_direct=psx if (2 * s + hf) in DIRECT_G else None)
            prev = cur

    nc.finalize()
    return nc


def _g0(h0):
    return np.where(h0 >= 0.0, h0 + 0.5, 1.0 / (1.0 + np.exp(-h0))).astype(np.float32)


def _q8(a):
    return np.asarray(a, np.float32).astype(F8).astype(np.float32)


def _prep(x, conv_w, conv_b, h0):
    x = np.asarray(x, np.float32)
    conv_w = np.asarray(conv_w, np.float32)
    conv_b = np.asarray(conv_b, np.float32)
    h0 = np.asarray(h0, np.float32)

    wd = conv_w[0:64] - conv_w[64:128]          # [64out, 64in, 3, 3]
    wh = conv_w[128:192]
    bd = conv_b[0:64] - conv_b[64:128]
    bh = conv_b[128:192]

    # stored weight grids: diff on x64 (x_hi rows) / x4 (x_lo16 rows),
    # ht on x1 / (1/16). lhsT[k, m] = w[m, k, tap]; tap row = r0, col = dc+1
    def w_hi(r0, dc):
        r, c = r0, dc + 1
        blk = np.zeros((64, 128), np.float32)
        blk[:, 0:64] = _q8(64.0 * wd[:, :, r, c]).T
        blk[:, 64:128] = _q8(wh[:, :, r, c]).T
        return blk

    def w_lo(r0, dc):
        r, c = r0, dc + 1
        blk = np.zeros((64, 128), np.float32)
        blk[:, 0:64] = _q8(64.0 * wd[:, :, r, c] - _q8(64.0 * wd[:, :, r, c])).T
        blk[:, 64:128] = _q8(wh[:, :, r, c] - _q8(wh[:, :, r, c])).T
        return blk

    def w4_hi(r0, dc):
        r, c = r0, dc + 1
        blk = np.zeros((64, 128), np.float32)
        blk[:, 0:64] = _q8(4.0 * wd[:, :, r, c]).T
        blk[:, 64:128] = _q8(wh[:, :, r, c] / 16.0).T
        return blk

    def w4_lo(r0, dc):
        r, c = r0, dc + 1
        blk = np.zeros((64, 128), np.float32)
        blk[:, 0:64] = _q8(4.0 * wd[:, :, r, c] - _q8(4.0 * wd[:, :, r, c])).T
        blk[:, 64:128] = _q8(wh[:, :, r, c] / 16.0 - _q8(wh[:, :, r, c] / 16.0)).T
        return blk

    bias_hi = np.concatenate([64.0 * bd, bh + 0.5])
    b1 = _q8(bias_hi)
    b2 = _q8(bias_hi - b1)
    bias_blk = np.zeros((64, 128), np.float32)
    bias_blk[0, :] = b1
    bias_blk[1, :] = b2

    def half_w(spec, grp):
        region, r0, wk = spec
        if region == "ONES":
            return bias_blk if grp == 0 else np.zeros((64, 128), np.float32)
        if region == "T1":      # [x_hi(c0); x_hi(c-1)]
            f = w_hi if wk == "hi" else w_lo
            return f(r0, 0) if grp == 0 else f(r0, -1)
        if region == "T2":      # [x_hi(c+1); x_lo16(c+1)]
            if grp == 0:
                return (w_hi if wk == "hi" else w_lo)(r0, 1)
            return (w4_hi if wk == "hi" else w4_lo)(r0, 1)
        # T3: [x_lo16(c0); x_lo16(c-1)], only 'hi'
        return w4_hi(r0, 0) if grp == 0 else w4_hi(r0, -1)

    wtt = np.zeros((128, NPL * 128), np.float32)
    for j, (pa, pb) in enumerate(DR_PLAN):
        for q, spec in enumerate((pa, pb)):
            blk = wtt[:, (2 * j + q) * 128:(2 * j + q + 1) * 128]
            blk[0:64, :] = half_w(spec, 0)
            blk[64:128, :] = half_w(spec, 1)
    wtt = wtt.astype(F8)

    x4 = x.reshape(B, T, C, H, W)
    g0f = _g0(h0)

    in_maps = []
    for c in range(8):
        b, half = c // 2, c % 2
        base = np.zeros((64, T, HP, WD32), np.float32)
        if half == 0:
            base[:, :, 1:18, :] = x4[b].transpose(1, 0, 2, 3)[:, :, 0:17, :]
        else:
            base[:, :, 0:17, :] = x4[b].transpose(1, 0, 2, 3)[:, :, 15:32, :]
        bhi = _q8(base)
        blo = _q8(16.0 * (base - bhi))

        def shift(a, dc):
            # variant[r, w] = a[r, w + dc], zeros shifted in at edges
            v = np.zeros_like(a)
            if dc == 0:
                v[:] = a
            elif dc == 1:
                v[:, :, :, 0:WD32 - 1] = a[:, :, :, 1:WD32]
            else:
                v[:, :, :, 1:WD32] = a[:, :, :, 0:WD32 - 1]
            return v

        def seg_blocks(top, bot):
            z = np.zeros((128, NSEG, SEGR), np.float32)
            z[0:64] = top.reshape(64, NSEG, SEGR)
            z[64:128] = bot.reshape(64, NSEG, SEGR)
            return z

        t1 = seg_blocks(shift(bhi, 0), shift(bhi, -1))
        t2 = seg_blocks(shift(bhi, 1), shift(blo, 1))
        t3 = seg_blocks(shift(blo, 0), shift(blo, -1))
        ones = np.ones((128, NSEG, RC), np.float32)
        xall = np.concatenate([t1, t2, t3, ones], axis=2)

        g0c = g0f[b, :, 16 * half:16 * half + 16, :].reshape(64, NPX)
        cstc = np.zeros((128, 2 + HPX), np.float32)
        cstc[0:64, 0] = 1.0 / 64.0
        cstc[64:128, 0] = 1.0
        cstc[0:64, 1] = 0.0
        cstc[64:128, 1] = -0.5
        cstc[0:64, 2:] = g0c[:, 0:HPX]
        cstc[64:128, 2:] = g0c[:, HPX:NPX]
        in_maps.append({
            "xs": xall.reshape(128, -1).astype(F8),
            "wt": wtt,
            "cst": cstc,
        })
    return in_maps


def _unpack(results):
    outf = np.empty((B, T, C, 32, 32), np.float32)
    for c in range(8):
        b, half = c // 2, c % 2
        arr = np.asarray(results[c]["out"], dtype=np.float32)  # [NSEG,128,NF2]
        hs = arr.reshape(NSEG, 2, 64, HPX, TS)[:, :, :, :, 1:]
        hs = hs.transpose(0, 4, 2, 1, 3).reshape(T, C, NPX)
        outf[b, :, :, 16 * half:16 * half + 16, :] = hs.reshape(T, C, 16, 32)
    return outf.reshape(B * T, C, 32, 32)


def kernel(x, conv_w, conv_b, h0):
    if "nc" not in _CACHE:
        _CACHE["nc"] = _build()
    nc = _CACHE["nc"]
    in_maps = _prep(x, conv_w, conv_b, h0)
    _CACHE["in_maps"] = in_maps
    res = run_bass_kernel_spmd(nc, in_maps, core_ids=list(range(8)))
    return _unpack(res.results)


# revision 43
# speedup vs baseline: 1.0011x; 1.0011x over previous

# Trainium2 Bass kernel for MinConvExpLSTMCell.
#
# Math (linear-space reformulation of the reference's log-space scan):
#   y = conv3x3(x, W) + b; [f_gate, i_gate, h_tilde] = split(y)
#   diff = f_gate - i_gate = conv(x, W_f - W_i) + (b_f - b_i)
#   f = sigmoid(diff);  i = 1 - f
#   g = max(sigmoid(ht), ht + 0.5)              (exact identity for g(ht))
#   h_t = f_t * h_{t-1} + i_t * g_t,  h_{-1} = g(h0)
#
# Sharding: 8 cores = 4 batches x 2 spatial halves (16 output rows each,
# 1 halo row; images stored 18 rows x 32 cols, vertical pad only).
#
# Conv: fp8 DoubleRow matmuls (0.5 cyc/row), 8 per timestep. Horizontal
# taps come from pre-shifted column-variant copies of the image so every
# window is a CONTIGUOUS 512-elem span (HW DR needs [K, 2, N] APs).
# Partition groups per region:
#   T1 = [x_hi(c0); x_hi(c-1)]   T2 = [x_hi(c+1); x_lo16(c+1)]
#   T3 = [x_lo16(c0); x_lo16(c-1)]   ONES (bias rows, Dekker fp8 pair)
# x_hi = fp8(x), x_lo16 = fp8(16(x - x_hi)). Weights are 2-term fp8
# Dekker pairs (w_hi + w_lo planes); diff out-channels on a x64 grid
# (fp8 subnormal avoidance), rescaled inside the sigmoid via a
# per-partition scale AP; ht at x1 (consumed raw).
#
# PSUM [64*(diff+bd); ht+bh+0.5] consumers: one full-width sigmoid on
# the scalar engine -> f (top), s (bottom); one Copy drain of the ht
# half -> YH. DVE: pixel-split i = 1-f (4x mode), F2 = 1-I2, fused
# split g = max(s, YH), and the per-pixel tensor_tensor_scan. gpsimd:
# u = i*g (in place on I2), scan-reset memset, h chain copy. All gates
# fp16. Scan layout pixel-major, time-minor, 9 slots/pixel with an f=0
# reset column chaining segments.

import sys
import numpy as np

sys.path.insert(0, "/opt/trn_rl_repo")

import ml_dtypes
from contextlib import ExitStack

import concourse.bass as bass
import bass_rust
import concourse.bacc as bacc
import concourse.mybir as mybir
from concourse.tile import TileContext
from concourse.bass_utils import run_bass_kernel_spmd

F8 = ml_dtypes.float8_e4m3
F16 = np.float16
B, T, C, H, W = 4, 64, 64, 32, 32
SEG = 8
NSEG = T // SEG
HP, WD32 = 18, 32          # padded rows x cols (no horizontal pad)
RC = HP * WD32             # 576
NPX = 16 * 32              # 512 output pixels per core
HPX = NPX // 2             # 256 pixels per partition-half
TS = SEG + 1               # 9 scan slots per pixel per segment
NF2 = HPX * TS             # 2304 pixel-split scan free size
# per-segment stream: [T1 | T2 | T3] (SEG*RC each) + ones block (RC)
SEGR = SEG * RC
SEGX = 3 * SEGR + RC
OFF_T1, OFF_T2, OFF_T3, OFF_ONES = 0, SEGR, 2 * SEGR, 3 * SEGR

# DR plan: 8 matmuls/timestep, each = (planeA, planeB); plane =
# (region, r0, wkind). wkind: 'hi'/'lo' = Dekker weight terms,
# 'b' = bias ones plane.
DR_PLAN = [
    (("T1", 0, "hi"), ("T1", 0, "lo")),
    (("T1", 1, "hi"), ("T1", 1, "lo")),
    (("T1", 2, "hi"), ("T1", 2, "lo")),
    (("T2", 0, "hi"), ("T2", 0, "lo")),
    (("T2", 1, "hi"), ("T2", 1, "lo")),
    (("T2", 2, "hi"), ("T2", 2, "lo")),
    (("T3", 0, "hi"), ("T3", 1, "hi")),
    (("T3", 2, "hi"), ("ONES", 0, "b")),
]
NPL = 16
DIRECT_G = set()        # (direct-PSUM g reads hurt; keep empty)

_CACHE = {}


def _planes(win, d):
    """Insert a [stride=d, count=2] plane dim after the partition dim."""
    c = win.copy()
    dims = [list(x) for x in win.ap]
    c.ap = bass_rust.VecI64Pair([dims[0]] + [[d, 2]] + dims[1:])
    return c


def _off(region, ti, r0):
    if region == "ONES":
        return OFF_ONES
    base = {"T1": OFF_T1, "T2": OFF_T2, "T3": OFF_T3}[region]
    return base + ti * RC + r0 * WD32


def _build():
    f32 = mybir.dt.float32
    f16 = mybir.dt.float16
    f8 = mybir.dt.float8e4
    AF = mybir.ActivationFunctionType
    OP = mybir.AluOpType
    DRM = mybir.MatmulPerfMode.DoubleRow

    nc = bacc.Bacc()
    xs = nc.dram_tensor("xs", [128, NSEG * SEGX], f8, kind="ExternalInput")
    wt = nc.dram_tensor("wt", [128, NPL * 128], f8, kind="ExternalInput")
    cst = nc.dram_tensor("cst", [128, 2 + HPX], f32, kind="ExternalInput")
    out = nc.dram_tensor("out", [NSEG, 128, NF2], f16, kind="ExternalOutput")

    with TileContext(nc) as tc, ExitStack() as ctx:
        cpool = ctx.enter_context(tc.tile_pool(name="consts", bufs=1))
        xpool = ctx.enter_context(tc.tile_pool(name="x", bufs=4))
        pspool = ctx.enter_context(tc.tile_pool(name="ps", bufs=2, space="PSUM"))
        sdpool = ctx.enter_context(tc.tile_pool(name="sd", bufs=3))
        yhpool = ctx.enter_context(tc.tile_pool(name="yh", bufs=3))
        fpool = ctx.enter_context(tc.tile_pool(name="f", bufs=3))
        ipool = ctx.enter_context(tc.tile_pool(name="i", bufs=3))
        gpool = ctx.enter_context(tc.tile_pool(name="g", bufs=3))
        hpool = ctx.enter_context(tc.tile_pool(name="h", bufs=3))

        w_sb = cpool.tile([128, NPL * 128], f8)
        nc.gpsimd.dma_start(w_sb[:, :], wt[:, :])
        cst_sb = cpool.tile([128, 2 + HPX], f32)
        nc.gpsimd.dma_start(cst_sb[:, :], cst[:, :])
        scalep = cst_sb[:, 0:1]            # [1/64 ; 1]
        biasp = cst_sb[:, 1:2]             # [0 ; -0.5]
        g0 = cst_sb[:, 2:2 + HPX]          # g(h0), pixel-split

        wv = w_sb.rearrange("p (j two m) -> p j two m", two=2, m=128)

        # preload the sigmoid act table while input DMAs stream
        warm = cpool.tile([128, 1], f16)
        nc.scalar.activation(warm[:, :], cst_sb[:, 0:1], AF.Sigmoid)

        def emit_matmuls(ps, xt, hf):
            for j, (pa, pb) in enumerate(DR_PLAN):
                lhsT = wv[:, j]
                for k in range(4):
                    ti = hf * 4 + k
                    oa = _off(pa[0], ti, pa[1])
                    ob = _off(pb[0], ti, pb[1])
                    win = xt[:, oa:oa + 512]
                    nc.tensor.matmul(
                        ps[:, k * 512:(k + 1) * 512], lhsT,
                        _planes(win, ob - oa),
                        start=(j == 0), stop=(j == len(DR_PLAN) - 1),
                        perf_mode=DRM)

        def emit_half_gates(tl, hf, yh_direct=None):
            (SDv, YHv, F2, I2, G2, F2v, I2v, G2v) = tl
            lo, hi = 4 * hf, 4 * hf + 4
            slo, shi = lo + 1, hi + 1
            # g = max(s, yh), dense read -> pixel-split write (DVE tt).
            # yh_direct: read ht straight from PSUM (no Act drain for
            # this half; slower DVE op but relieves the scalar engine).
            # g goes first on the DVE queue so PSUM frees promptly.
            if yh_direct is not None:
                ya = yh_direct[64:128, 0:HPX, :]
                yb = yh_direct[64:128, HPX:NPX, :]
            else:
                ya = YHv[64:128, 0:HPX, lo:hi]
                yb = YHv[64:128, HPX:NPX, lo:hi]
            nc.vector.tensor_tensor(
                G2v[0:64, :, lo:hi], SDv[64:128, 0:HPX, lo:hi], ya, OP.max)
            nc.vector.tensor_tensor(
                G2v[64:128, :, lo:hi], SDv[64:128, HPX:NPX, lo:hi], yb,
                OP.max)
            # i = 1 - f, pixel-split (DVE ts 4x mode)
            nc.vector.tensor_scalar(
                I2v[0:64, :, slo:shi], SDv[0:64, 0:HPX, lo:hi], -1.0, 1.0,
                OP.mult, OP.add)
            nc.vector.tensor_scalar(
                I2v[64:128, :, slo:shi], SDv[0:64, HPX:NPX, lo:hi], -1.0, 1.0,
                OP.mult, OP.add)
            # f = 1 - i, full width from I2 (DVE ts 4x mode)
            nc.vector.tensor_scalar(
                F2v[:, :, slo:shi], I2v[:, :, slo:shi], -1.0, 1.0,
                OP.mult, OP.add)
            # u = i * g, in place on I2 (gpsimd tt mult)
            nc.gpsimd.tensor_tensor(
                I2v[:, :, slo:shi], I2v[:, :, slo:shi], G2v[:, :, lo:hi],
                OP.mult)

        def emit_scan(tl, s_idx):
            (SDv, YHv, F2, I2, G2, F2v, I2v, G2v) = tl
            H2 = hpool.tile([128, NF2], f16)
            nc.vector.tensor_tensor_scan(
                H2[:, :], F2[:, :], I2[:, :], 0.0, OP.mult, OP.add)
            nc.gpsimd.dma_start(out[s_idx], H2[:, :])
            return H2

        def fetch_x(s):
            # T1+T2 on the SP queue, T3+ones on the gpsimd queue: the
            # sim charges DMA transfer time to the issuing queue, so
            # splitting the byte volume matters more than issue count
            xt = xpool.tile([128, SEGX], f8)
            o = s * SEGX
            nc.sync.dma_start(xt[:, 0:SEGR], xs[:, o:o + SEGR])
            nc.gpsimd.dma_start(
                xt[:, SEGR:2 * SEGR], xs[:, o + SEGR:o + 2 * SEGR])
            nc.sync.dma_start(
                xt[:, 2 * SEGR:SEGX], xs[:, o + 2 * SEGR:o + SEGX])
            return xt

        prev = None
        h_prev = None
        xt_next = fetch_x(0)

        for s in range(NSEG):
            xt = xt_next
            if s + 1 < NSEG:
                xt_next = fetch_x(s + 1)

            SD = sdpool.tile([128, NPX * SEG], f16)       # f top / s bottom
            SDv = SD.rearrange("p (px t) -> p px t", t=SEG)
            YH = yhpool.tile([128, NPX * SEG], f16)       # ht+bh+0.5 (bottom)
            YHv = YH.rearrange("p (px t) -> p px t", t=SEG)
            F2 = fpool.tile([128, NF2], f16)
            I2 = ipool.tile([128, NF2], f16)
            G2 = gpool.tile([128, HPX * SEG], f16)
            cur = (SDv, YHv, F2, I2, G2,
                   F2.rearrange("p (px t) -> p px t", t=TS),
                   I2.rearrange("p (px t) -> p px t", t=TS),
                   G2.rearrange("p (px t) -> p px t", t=SEG))

            # reset column for the per-pixel scan chains
            nc.gpsimd.memset(cur[5][:, :, 0], 0.0)

            for hf in range(2):
                ps = pspool.tile([128, 4 * 512], f32)
                psx = ps.rearrange("p (k x) -> p x k", k=4)
                lo, hi = 4 * hf, 4 * hf + 4
                emit_matmuls(ps, xt, hf)
                last_half = (s == NSEG - 1 and hf == 1)
                if not last_half:
                    # f|s = sigmoid(psum * [1/64;1] + [0;-0.5]), full width
                    nc.scalar.activation(
                        SDv[:, :, lo:hi], psx[:, :, :], AF.Sigmoid,
                        bias=biasp, scale=scalep)
                    # drain ht+bh+0.5 for g's linear branch
                    nc.scalar.activation(
                        YHv[64:128, :, lo:hi], psx[64:128, :, :], AF.Copy)
                if hf == 0:
                    # u col0 = h_{-1} (chains segments)
                    if prev is not None:
                        h_prev = emit_scan(prev, s - 1)
                        hp_px = h_prev.rearrange("p (px t) -> p px t", t=TS)
                        nc.gpsimd.tensor_scalar(
                            cur[6][:, :, 0], hp_px[:, :, SEG], 1.0, 0.0,
                            OP.mult, OP.add)
                    else:
                        nc.gpsimd.tensor_scalar(
                            cur[6][:, :, 0], g0, 1.0, 0.0, OP.mult, OP.add)
                if last_half:
                    # last half: pixel-quarter pipeline so the tail
                    # overlaps sigmoid/drain/gates/scan/DMA across engines
                    (SDv2, YHv2, F2_, I2_, G2_, F2v2, I2v2, G2v2) = cur
                    H2t = hpool.tile([128, NF2], f16)
                    edges = [0, 64, 128, 192, 256]
                    for q in range(4):
                        e0, e1 = edges[q], edges[q + 1]
                        A = slice(e0, e1)
                        A2 = slice(256 + e0, 256 + e1)
                        pq = slice(e0, e1)
                        for dr in (A, A2):
                            nc.scalar.activation(
                                SDv2[:, dr, lo:hi], psx[:, dr, :],
                                AF.Sigmoid, bias=biasp, scale=scalep)
                            nc.scalar.activation(
                                YHv2[64:128, dr, lo:hi], psx[64:128, dr, :],
                                AF.Copy)
                        nc.vector.tensor_scalar(
                            I2v2[0:64, pq, lo + 1:hi + 1],
                            SDv2[0:64, A, lo:hi], -1.0, 1.0, OP.mult, OP.add)
                        nc.vector.tensor_scalar(
                            I2v2[64:128, pq, lo + 1:hi + 1],
                            SDv2[0:64, A2, lo:hi], -1.0, 1.0, OP.mult, OP.add)
                        nc.vector.tensor_scalar(
                            F2v2[:, pq, lo + 1:hi + 1],
                            I2v2[:, pq, lo + 1:hi + 1], -1.0, 1.0,
                            OP.mult, OP.add)
                        nc.vector.tensor_tensor(
                            G2v2[0:64, pq, lo:hi], SDv2[64:128, A, lo:hi],
                            YHv2[64:128, A, lo:hi], OP.max)
                        nc.vector.tensor_tensor(
                            G2v2[64:128, pq, lo:hi], SDv2[64:128, A2, lo:hi],
                            YHv2[64:128, A2, lo:hi], OP.max)
                        nc.gpsimd.tensor_tensor(
                            I2v2[:, pq, lo + 1:hi + 1],
                            I2v2[:, pq, lo + 1:hi + 1],
                            G2v2[:, pq, lo:hi], OP.mult)
                        lo2, hi2 = e0 * TS, e1 * TS
                        nc.vector.tensor_tensor_scan(
                            H2t[:, lo2:hi2], F2_[:, lo2:hi2], I2_[:, lo2:hi2],
                            0.0, OP.mult, OP.add)
                        nc.sync.dma_start(
                            out[NSEG - 1, :, lo2:hi2], H2t[:, lo2:hi2])
                else:
                    emit_half_gates(
                        cur, hf,
                        yh_direct=psx if (2 * s + hf) in DIRECT_G else None)
            prev = cur

    nc.finalize()
    return nc


def _g0(h0):
    return np.where(h0 >= 0.0, h0 + 0.5, 1.0 / (1.0 + np.exp(-h0))).astype(np.float32)


def _q8(a):
    return np.asarray(a, np.float32).astype(F8).astype(np.float32)


def _prep(x, conv_w, conv_b, h0):
    x = np.asarray(x, np.float32)
    conv_w = np.asarray(conv_w, np.float32)
    conv_b = np.asarray(conv_b, np.float32)
    h0 = np.asarray(h0, np.float32)

    wd = conv_w[0:64] - conv_w[64:128]          # [64out, 64in, 3, 3]
    wh = conv_w[128:192]
    bd = conv_b[0:64] - conv_b[64:128]
    bh = conv_b[128:192]

    # stored weight grids: diff on x64 (x_hi rows) / x4 (x_lo16 rows),
    # ht on x1 / (1/16). lhsT[k, m] = w[m, k, tap]; tap row = r0, col = dc+1
    def w_hi(r0, dc):
        r, c = r0, dc + 1
        blk = np.zeros((64, 128), np.float32)
        blk[:, 0:64] = _q8(64.0 * wd[:, :, r, c]).T
        blk[:, 64:128] = _q8(wh[:, :, r, c]).T
        return blk

    def w_lo(r0, dc):
        r, c = r0, dc + 1
        blk = np.zeros((64, 128), np.float32)
        blk[:, 0:64] = _q8(64.0 * wd[:, :, r, c] - _q8(64.0 * wd[:, :, r, c])).T
        blk[:, 64:128] = _q8(wh[:, :, r, c] - _q8(wh[:, :, r, c])).T
        return blk

    def w4_hi(r0, dc):
        r, c = r0, dc + 1
        blk = np.zeros((64, 128), np.float32)
        blk[:, 0:64] = _q8(4.0 * wd[:, :, r, c]).T
        blk[:, 64:128] = _q8(wh[:, :, r, c] / 16.0).T
        return blk

    def w4_lo(r0, dc):
        r, c = r0, dc + 1
        blk = np.zeros((64, 128), np.float32)
        blk[:, 0:64] = _q8(4.0 * wd[:, :, r, c] - _q8(4.0 * wd[:, :, r, c])).T
        blk[:, 64:128] = _q8(wh[:, :, r, c] / 16.0 - _q8(wh[:, :, r, c] / 16.0)).T
        return blk

    bias_hi = np.concatenate([64.0 * bd, bh + 0.5])
    b1 = _q8(bias_hi)
    b2 = _q8(bias_hi - b1)
    bias_blk = np.zeros((64, 128), np.float32)
    bias_blk[0, :] = b1
    bias_blk[1, :] = b2

    def half_w(spec, grp):
        region, r0, wk = spec
        if region == "ONES":
            return bias_blk if grp == 0 else np.zeros((64, 128), np.float32)
        if region == "T1":      # [x_hi(c0); x_hi(c-1)]
            f = w_hi if wk == "hi" else w_lo
            return f(r0, 0) if grp == 0 else f(r0, -1)
        if region == "T2":      # [x_hi(c+1); x_lo16(c+1)]
            if grp == 0:
                return (w_hi if wk == "hi" else w_lo)(r0, 1)
            return (w4_hi if wk == "hi" else w4_lo)(r0, 1)
        # T3: [x_lo16(c0); x_lo16(c-1)], only 'hi'
        return w4_hi(r0, 0) if grp == 0 else w4_hi(r0, -1)

    wtt = np.zeros((128, NPL * 128), np.float32)
    for j, (pa, pb) in enumerate(DR_PLAN):
        for q, spec in enumerate((pa, pb)):
            blk = wtt[:, (2 * j + q) * 128:(2 * j + q + 1) * 128]
            blk[0:64, :] = half_w(spec, 0)
            blk[64:128, :] = half_w(spec, 1)
    wtt = wtt.astype(F8)

    x4 = x.reshape(B, T, C, H, W)
    g0f = _g0(h0)

    in_maps = []
    for c in range(8):
        b, half = c // 2, c % 2
        base = np.zeros((64, T, HP, WD32), np.float32)
        if half == 0:
            base[:, :, 1:18, :] = x4[b].transpose(1, 0, 2, 3)[:, :, 0:17, :]
        else:
            base[:, :, 0:17, :] = x4[b].transpose(1, 0, 2, 3)[:, :, 15:32, :]
        bhi = _q8(base)
        blo = _q8(16.0 * (base - bhi))

        def shift(a, dc):
            # variant[r, w] = a[r, w + dc], zeros shifted in at edges
            v = np.zeros_like(a)
            if dc == 0:
                v[:] = a
            elif dc == 1:
                v[:, :, :, 0:WD32 - 1] = a[:, :, :, 1:WD32]
            else:
                v[:, :, :, 1:WD32] = a[:, :, :, 0:WD32 - 1]
            return v

        def seg_blocks(top, bot):
            z = np.zeros((128, NSEG, SEGR), np.float32)
            z[0:64] = top.reshape(64, NSEG, SEGR)
            z[64:128] = bot.reshape(64, NSEG, SEGR)
            return z

        t1 = seg_blocks(shift(bhi, 0), shift(bhi, -1))
        t2 = seg_blocks(shift(bhi, 1), shift(blo, 1))
        t3 = seg_blocks(shift(blo, 0), shift(blo, -1))
        ones = np.ones((128, NSEG, RC), np.float32)
        xall = np.concatenate([t1, t2, t3, ones], axis=2)

        g0c = g0f[b, :, 16 * half:16 * half + 16, :].reshape(64, NPX)
        cstc = np.zeros((128, 2 + HPX), np.float32)
        cstc[0:64, 0] = 1.0 / 64.0
        cstc[64:128, 0] = 1.0
        cstc[0:64, 1] = 0.0
        cstc[64:128, 1] = -0.5
        cstc[0:64, 2:] = g0c[:, 0:HPX]
        cstc[64:128, 2:] = g0c[:, HPX:NPX]
        in_maps.append({
            "xs": xall.reshape(128, -1).astype(F8),
            "wt": wtt,
            "cst": cstc,
        })
    return in_maps


def _unpack(results):
    outf = np.empty((B, T, C, 32, 32), np.float32)
    for c in range(8):
        b, half = c // 2, c % 2
        arr = np.asarray(results[c]["out"], dtype=np.float32)  # [NSEG,128,NF2]
        hs = arr.reshape(NSEG, 2, 64, HPX, TS)[:, :, :, :, 1:]
        hs = hs.transpose(0, 4, 2, 1, 3).reshape(T, C, NPX)
        outf[b, :, :, 16 * half:16 * half + 16, :] = hs.reshape(T, C, 16, 32)
    return outf.reshape(B * T, C, 32, 32)


def kernel(x, conv_w, conv_b, h0):
    if "nc" not in _CACHE:
        _CACHE["nc"] = _build()
    nc = _CACHE["nc"]
    in_maps = _prep(x, conv_w, conv_b, h0)
    _CACHE["in_maps"] = in_maps
    res = run_bass_kernel_spmd(nc, in_maps, core_ids=list(range(8)))
    return _unpack(res.results)


# revision 49
# speedup vs baseline: 1.0090x; 1.0079x over previous

# Trainium2 Bass kernel for MinConvExpLSTMCell.
#
# Math (linear-space reformulation of the reference's log-space scan):
#   y = conv3x3(x, W) + b; [f_gate, i_gate, h_tilde] = split(y)
#   diff = f_gate - i_gate = conv(x, W_f - W_i) + (b_f - b_i)
#   f = sigmoid(diff);  i = 1 - f
#   g = max(sigmoid(ht), ht + 0.5)              (exact identity for g(ht))
#   h_t = f_t * h_{t-1} + i_t * g_t,  h_{-1} = g(h0)
#
# Sharding: 8 cores = 4 batches x 2 spatial halves (16 output rows each,
# 1 halo row; images stored 18 rows x 32 cols, vertical pad only).
#
# Conv: fp8 DoubleRow matmuls (0.5 cyc/row), 8 per timestep. Horizontal
# taps come from pre-shifted column-variant copies of the image so every
# window is a CONTIGUOUS 512-elem span (HW DR needs [K, 2, N] APs).
# Partition groups per region:
#   T1 = [x_hi(c0); x_hi(c-1)]   T2 = [x_hi(c+1); x_lo16(c+1)]
#   T3 = [x_lo16(c0); x_lo16(c-1)]   ONES (bias rows, Dekker fp8 pair)
# x_hi = fp8(x), x_lo16 = fp8(16(x - x_hi)). Weights are 2-term fp8
# Dekker pairs (w_hi + w_lo planes); diff out-channels on a x64 grid
# (fp8 subnormal avoidance), rescaled inside the sigmoid via a
# per-partition scale AP; ht at x1 (consumed raw).
#
# PSUM [64*(diff+bd); ht+bh+0.5] consumers: one full-width sigmoid on
# the scalar engine -> f (top), s (bottom); one Copy drain of the ht
# half -> YH. DVE: pixel-split i = 1-f (4x mode), F2 = 1-I2, fused
# split g = max(s, YH), and the per-pixel tensor_tensor_scan. gpsimd:
# u = i*g (in place on I2), scan-reset memset, h chain copy. All gates
# fp16. Scan layout pixel-major, time-minor, 9 slots/pixel with an f=0
# reset column chaining segments.

import sys
import numpy as np

sys.path.insert(0, "/opt/trn_rl_repo")

import ml_dtypes
from contextlib import ExitStack

import concourse.bass as bass
import bass_rust
import concourse.bacc as bacc
import concourse.mybir as mybir
from concourse.tile import TileContext
from concourse.bass_utils import run_bass_kernel_spmd

F8 = ml_dtypes.float8_e4m3
F16 = np.float16
B, T, C, H, W = 4, 64, 64, 32, 32
SEG = 8
NSEG = T // SEG
HP, WD32 = 18, 32          # padded rows x cols (no horizontal pad)
RC = HP * WD32             # 576
NPX = 16 * 32              # 512 output pixels per core
HPX = NPX // 2             # 256 pixels per partition-half
TS = SEG + 1               # 9 scan slots per pixel per segment
NF2 = HPX * TS             # 2304 pixel-split scan free size
# per-segment stream: [T1 | T2 | T3] (SEG*RC each) + ones block (RC)
SEGR = SEG * RC
SEGX = 3 * SEGR + RC
OFF_T1, OFF_T2, OFF_T3, OFF_ONES = 0, SEGR, 2 * SEGR, 3 * SEGR

# DR plan: 8 matmuls/timestep, each = (planeA, planeB); plane =
# (region, r0, wkind). wkind: 'hi'/'lo' = Dekker weight terms,
# 'b' = bias ones plane.
DR_PLAN = [
    (("T1", 0, "hi"), ("T1", 0, "lo")),
    (("T1", 1, "hi"), ("T1", 1, "lo")),
    (("T1", 2, "hi"), ("T1", 2, "lo")),
    (("T2", 0, "hi"), ("T2", 0, "lo")),
    (("T2", 1, "hi"), ("T2", 1, "lo")),
    (("T2", 2, "hi"), ("T2", 2, "lo")),
    (("T3", 0, "hi"), ("T3", 1, "hi")),
    (("T3", 2, "hi"), ("ONES", 0, "b")),
]
NPL = 16
DIRECT_G = set()        # (direct-PSUM g reads hurt; keep empty)

_CACHE = {}


def _planes(win, d):
    """Insert a [stride=d, count=2] plane dim after the partition dim."""
    c = win.copy()
    dims = [list(x) for x in win.ap]
    c.ap = bass_rust.VecI64Pair([dims[0]] + [[d, 2]] + dims[1:])
    return c


def _off(region, ti, r0):
    if region == "ONES":
        return OFF_ONES
    base = {"T1": OFF_T1, "T2": OFF_T2, "T3": OFF_T3}[region]
    return base + ti * RC + r0 * WD32


def _build():
    f32 = mybir.dt.float32
    f16 = mybir.dt.float16
    f8 = mybir.dt.float8e4
    AF = mybir.ActivationFunctionType
    OP = mybir.AluOpType
    DRM = mybir.MatmulPerfMode.DoubleRow

    nc = bacc.Bacc()
    xs = nc.dram_tensor("xs", [128, NSEG * SEGX], f8, kind="ExternalInput")
    wt = nc.dram_tensor("wt", [128, NPL * 128], f8, kind="ExternalInput")
    cst = nc.dram_tensor("cst", [128, 2 + HPX], f32, kind="ExternalInput")
    out = nc.dram_tensor("out", [NSEG, 128, NF2], f16, kind="ExternalOutput")

    with TileContext(nc) as tc, ExitStack() as ctx:
        cpool = ctx.enter_context(tc.tile_pool(name="consts", bufs=1))
        xpool = ctx.enter_context(tc.tile_pool(name="x", bufs=4))
        pspool = ctx.enter_context(tc.tile_pool(name="ps", bufs=2, space="PSUM"))
        sdpool = ctx.enter_context(tc.tile_pool(name="sd", bufs=3))
        yhpool = ctx.enter_context(tc.tile_pool(name="yh", bufs=3))
        fpool = ctx.enter_context(tc.tile_pool(name="f", bufs=3))
        ipool = ctx.enter_context(tc.tile_pool(name="i", bufs=3))
        gpool = ctx.enter_context(tc.tile_pool(name="g", bufs=3))
        hpool = ctx.enter_context(tc.tile_pool(name="h", bufs=3))

        w_sb = cpool.tile([128, NPL * 128], f8)
        nc.gpsimd.dma_start(w_sb[:, :], wt[:, :])
        cst_sb = cpool.tile([128, 2 + HPX], f32)
        nc.gpsimd.dma_start(cst_sb[:, :], cst[:, :])
        scalep = cst_sb[:, 0:1]            # [1/64 ; 1]
        biasp = cst_sb[:, 1:2]             # [0 ; -0.5]
        g0 = cst_sb[:, 2:2 + HPX]          # g(h0), pixel-split

        wv = w_sb.rearrange("p (j two m) -> p j two m", two=2, m=128)

        # preload the sigmoid act table while input DMAs stream
        warm = cpool.tile([128, 1], f16)
        nc.scalar.activation(warm[:, :], cst_sb[:, 0:1], AF.Sigmoid)

        def emit_matmuls(ps, xt, hf):
            for j, (pa, pb) in enumerate(DR_PLAN):
                lhsT = wv[:, j]
                for k in range(4):
                    ti = hf * 4 + k
                    oa = _off(pa[0], ti, pa[1])
                    ob = _off(pb[0], ti, pb[1])
                    win = xt[:, oa:oa + 512]
                    nc.tensor.matmul(
                        ps[:, k * 512:(k + 1) * 512], lhsT,
                        _planes(win, ob - oa),
                        start=(j == 0), stop=(j == len(DR_PLAN) - 1),
                        perf_mode=DRM)

        def emit_half_gates(tl, hf, yh_direct=None):
            (SDv, YHv, F2, I2, G2, F2v, I2v, G2v) = tl
            lo, hi = 4 * hf, 4 * hf + 4
            slo, shi = lo + 1, hi + 1
            # g = max(s, yh), dense read -> pixel-split write (DVE tt).
            # yh_direct: read ht straight from PSUM (no Act drain for
            # this half; slower DVE op but relieves the scalar engine).
            # g goes first on the DVE queue so PSUM frees promptly.
            if yh_direct is not None:
                ya = yh_direct[64:128, 0:HPX, :]
                yb = yh_direct[64:128, HPX:NPX, :]
            else:
                ya = YHv[64:128, 0:HPX, lo:hi]
                yb = YHv[64:128, HPX:NPX, lo:hi]
            nc.vector.tensor_tensor(
                G2v[0:64, :, lo:hi], SDv[64:128, 0:HPX, lo:hi], ya, OP.max)
            nc.vector.tensor_tensor(
                G2v[64:128, :, lo:hi], SDv[64:128, HPX:NPX, lo:hi], yb,
                OP.max)
            # i = 1 - f, pixel-split (DVE ts 4x mode)
            nc.vector.tensor_scalar(
                I2v[0:64, :, slo:shi], SDv[0:64, 0:HPX, lo:hi], -1.0, 1.0,
                OP.mult, OP.add)
            nc.vector.tensor_scalar(
                I2v[64:128, :, slo:shi], SDv[0:64, HPX:NPX, lo:hi], -1.0, 1.0,
                OP.mult, OP.add)
            # f = 1 - i, full width from I2 (DVE ts 4x mode)
            nc.vector.tensor_scalar(
                F2v[:, :, slo:shi], I2v[:, :, slo:shi], -1.0, 1.0,
                OP.mult, OP.add)
            # u = i * g, in place on I2 (gpsimd tt mult)
            nc.gpsimd.tensor_tensor(
                I2v[:, :, slo:shi], I2v[:, :, slo:shi], G2v[:, :, lo:hi],
                OP.mult)

        def emit_scan(tl, s_idx):
            (SDv, YHv, F2, I2, G2, F2v, I2v, G2v) = tl
            H2 = hpool.tile([128, NF2], f16)
            nc.vector.tensor_tensor_scan(
                H2[:, :], F2[:, :], I2[:, :], 0.0, OP.mult, OP.add)
            # last regular scan's DMA on SP so it can't queue ahead of
            # the tail's gpsimd u ops
            eng = nc.sync if s_idx == NSEG - 2 else nc.gpsimd
            eng.dma_start(out[s_idx], H2[:, :])
            return H2

        def fetch_x(s):
            # T1+T2 on the SP queue, T3+ones on the gpsimd queue: the
            # sim charges DMA transfer time to the issuing queue, so
            # splitting the byte volume matters more than issue count
            xt = xpool.tile([128, SEGX], f8)
            o = s * SEGX
            nc.sync.dma_start(xt[:, 0:SEGR], xs[:, o:o + SEGR])
            nc.gpsimd.dma_start(
                xt[:, SEGR:2 * SEGR], xs[:, o + SEGR:o + 2 * SEGR])
            nc.sync.dma_start(
                xt[:, 2 * SEGR:SEGX], xs[:, o + 2 * SEGR:o + SEGX])
            return xt

        prev = None
        h_prev = None
        xt_next = fetch_x(0)

        for s in range(NSEG):
            xt = xt_next
            if s + 1 < NSEG:
                xt_next = fetch_x(s + 1)

            SD = sdpool.tile([128, NPX * SEG], f16)       # f top / s bottom
            SDv = SD.rearrange("p (px t) -> p px t", t=SEG)
            YH = yhpool.tile([128, NPX * SEG], f16)       # ht+bh+0.5 (bottom)
            YHv = YH.rearrange("p (px t) -> p px t", t=SEG)
            F2 = fpool.tile([128, NF2], f16)
            I2 = ipool.tile([128, NF2], f16)
            G2 = gpool.tile([128, HPX * SEG], f16)
            cur = (SDv, YHv, F2, I2, G2,
                   F2.rearrange("p (px t) -> p px t", t=TS),
                   I2.rearrange("p (px t) -> p px t", t=TS),
                   G2.rearrange("p (px t) -> p px t", t=SEG))

            # reset column for the per-pixel scan chains
            nc.gpsimd.memset(cur[5][:, :, 0], 0.0)

            for hf in range(2):
                ps = pspool.tile([128, 4 * 512], f32)
                psx = ps.rearrange("p (k x) -> p x k", k=4)
                lo, hi = 4 * hf, 4 * hf + 4
                emit_matmuls(ps, xt, hf)
                last_half = (s == NSEG - 1 and hf == 1)
                if not last_half:
                    # f|s = sigmoid(psum * [1/64;1] + [0;-0.5]), full width
                    nc.scalar.activation(
                        SDv[:, :, lo:hi], psx[:, :, :], AF.Sigmoid,
                        bias=biasp, scale=scalep)
                    # drain ht+bh+0.5 for g's linear branch
                    nc.scalar.activation(
                        YHv[64:128, :, lo:hi], psx[64:128, :, :], AF.Copy)
                if hf == 0:
                    # u col0 = h_{-1} (chains segments)
                    if prev is not None:
                        h_prev = emit_scan(prev, s - 1)
                        hp_px = h_prev.rearrange("p (px t) -> p px t", t=TS)
                        nc.gpsimd.tensor_scalar(
                            cur[6][:, :, 0], hp_px[:, :, SEG], 1.0, 0.0,
                            OP.mult, OP.add)
                    else:
                        nc.gpsimd.tensor_scalar(
                            cur[6][:, :, 0], g0, 1.0, 0.0, OP.mult, OP.add)
                if last_half:
                    # last half: pixel-quarter pipeline so the tail
                    # overlaps sigmoid/drain/gates/scan/DMA across engines
                    (SDv2, YHv2, F2_, I2_, G2_, F2v2, I2v2, G2v2) = cur
                    H2t = hpool.tile([128, NF2], f16)
                    edges = [0, 110, 200, 256]
                    for q in range(3):
                        e0, e1 = edges[q], edges[q + 1]
                        A = slice(e0, e1)
                        A2 = slice(256 + e0, 256 + e1)
                        pq = slice(e0, e1)
                        for dr in (A, A2):
                            nc.scalar.activation(
                                SDv2[:, dr, lo:hi], psx[:, dr, :],
                                AF.Sigmoid, bias=biasp, scale=scalep)
                            nc.scalar.activation(
                                YHv2[64:128, dr, lo:hi], psx[64:128, dr, :],
                                AF.Copy)
                        nc.vector.tensor_scalar(
                            I2v2[0:64, pq, lo + 1:hi + 1],
                            SDv2[0:64, A, lo:hi], -1.0, 1.0, OP.mult, OP.add)
                        nc.vector.tensor_scalar(
                            I2v2[64:128, pq, lo + 1:hi + 1],
                            SDv2[0:64, A2, lo:hi], -1.0, 1.0, OP.mult, OP.add)
                        nc.vector.tensor_scalar(
                            F2v2[:, pq, lo + 1:hi + 1],
                            I2v2[:, pq, lo + 1:hi + 1], -1.0, 1.0,
                            OP.mult, OP.add)
                        nc.vector.tensor_tensor(
                            G2v2[0:64, pq, lo:hi], SDv2[64:128, A, lo:hi],
                            YHv2[64:128, A, lo:hi], OP.max)
                        nc.vector.tensor_tensor(
                            G2v2[64:128, pq, lo:hi], SDv2[64:128, A2, lo:hi],
                            YHv2[64:128, A2, lo:hi], OP.max)
                        nc.gpsimd.tensor_tensor(
                            I2v2[:, pq, lo + 1:hi + 1],
                            I2v2[:, pq, lo + 1:hi + 1],
                            G2v2[:, pq, lo:hi], OP.mult)
                        lo2, hi2 = e0 * TS, e1 * TS
                        nc.vector.tensor_tensor_scan(
                            H2t[:, lo2:hi2], F2_[:, lo2:hi2], I2_[:, lo2:hi2],
                            0.0, OP.mult, OP.add)
                        nc.sync.dma_start(
                            out[NSEG - 1, :, lo2:hi2], H2t[:, lo2:hi2])
                else:
                    emit_half_gates(
                        cur, hf,
                        yh_direct=psx if (2 * s + hf) in DIRECT_G else None)
            prev = cur

    nc.finalize()
    return nc


def _g0(h0):
    return np.where(h0 >= 0.0, h0 + 0.5, 1.0 / (1.0 + np.exp(-h0))).astype(np.float32)


def _q8(a):
    return np.asarray(a, np.float32).astype(F8).astype(np.float32)


def _prep(x, conv_w, conv_b, h0):
    x = np.asarray(x, np.float32)
    conv_w = np.asarray(conv_w, np.float32)
    conv_b = np.asarray(conv_b, np.float32)
    h0 = np.asarray(h0, np.float32)

    wd = conv_w[0:64] - conv_w[64:128]          # [64out, 64in, 3, 3]
    wh = conv_w[128:192]
    bd = conv_b[0:64] - conv_b[64:128]
    bh = conv_b[128:192]

    # stored weight grids: diff on x64 (x_hi rows) / x4 (x_lo16 rows),
    # ht on x1 / (1/16). lhsT[k, m] = w[m, k, tap]; tap row = r0, col = dc+1
    def w_hi(r0, dc):
        r, c = r0, dc + 1
        blk = np.zeros((64, 128), np.float32)
        blk[:, 0:64] = _q8(64.0 * wd[:, :, r, c]).T
        blk[:, 64:128] = _q8(wh[:, :, r, c]).T
        return blk

    def w_lo(r0, dc):
        r, c = r0, dc + 1
        blk = np.zeros((64, 128), np.float32)
        blk[:, 0:64] = _q8(64.0 * wd[:, :, r, c] - _q8(64.0 * wd[:, :, r, c])).T
        blk[:, 64:128] = _q8(wh[:, :, r, c] - _q8(wh[:, :, r, c])).T
        return blk

    def w4_hi(r0, dc):
        r, c = r0, dc + 1
        blk = np.zeros((64, 128), np.float32)
        blk[:, 0:64] = _q8(4.0 * wd[:, :, r, c]).T
        blk[:, 64:128] = _q8(wh[:, :, r, c] / 16.0).T
        return blk

    def w4_lo(r0, dc):
        r, c = r0, dc + 1
        blk = np.zeros((64, 128), np.float32)
        blk[:, 0:64] = _q8(4.0 * wd[:, :, r, c] - _q8(4.0 * wd[:, :, r, c])).T
        blk[:, 64:128] = _q8(wh[:, :, r, c] / 16.0 - _q8(wh[:, :, r, c] / 16.0)).T
        return blk

    bias_hi = np.concatenate([64.0 * bd, bh + 0.5])
    b1 = _q8(bias_hi)
    b2 = _q8(bias_hi - b1)
    bias_blk = np.zeros((64, 128), np.float32)
    bias_blk[0, :] = b1
    bias_blk[1, :] = b2

    def half_w(spec, grp):
        region, r0, wk = spec
        if region == "ONES":
            return bias_blk if grp == 0 else np.zeros((64, 128), np.float32)
        if region == "T1":      # [x_hi(c0); x_hi(c-1)]
            f = w_hi if wk == "hi" else w_lo
            return f(r0, 0) if grp == 0 else f(r0, -1)
        if region == "T2":      # [x_hi(c+1); x_lo16(c+1)]
            if grp == 0:
                return (w_hi if wk == "hi" else w_lo)(r0, 1)
            return (w4_hi if wk == "hi" else w4_lo)(r0, 1)
        # T3: [x_lo16(c0); x_lo16(c-1)], only 'hi'
        return w4_hi(r0, 0) if grp == 0 else w4_hi(r0, -1)

    wtt = np.zeros((128, NPL * 128), np.float32)
    for j, (pa, pb) in enumerate(DR_PLAN):
        for q, spec in enumerate((pa, pb)):
            blk = wtt[:, (2 * j + q) * 128:(2 * j + q + 1) * 128]
            blk[0:64, :] = half_w(spec, 0)
            blk[64:128, :] = half_w(spec, 1)
    wtt = wtt.astype(F8)

    x4 = x.reshape(B, T, C, H, W)
    g0f = _g0(h0)

    in_maps = []
    for c in range(8):
        b, half = c // 2, c % 2
        base = np.zeros((64, T, HP, WD32), np.float32)
        if half == 0:
            base[:, :, 1:18, :] = x4[b].transpose(1, 0, 2, 3)[:, :, 0:17, :]
        else:
            base[:, :, 0:17, :] = x4[b].transpose(1, 0, 2, 3)[:, :, 15:32, :]
        bhi = _q8(base)
        blo = _q8(16.0 * (base - bhi))

        def shift(a, dc):
            # variant[r, w] = a[r, w + dc], zeros shifted in at edges
            v = np.zeros_like(a)
            if dc == 0:
                v[:] = a
            elif dc == 1:
                v[:, :, :, 0:WD32 - 1] = a[:, :, :, 1:WD32]
            else:
                v[:, :, :, 1:WD32] = a[:, :, :, 0:WD32 - 1]
            return v

        def seg_blocks(top, bot):
            z = np.zeros((128, NSEG, SEGR), np.float32)
            z[0:64] = top.reshape(64, NSEG, SEGR)
            z[64:128] = bot.reshape(64, NSEG, SEGR)
            return z

        t1 = seg_blocks(shift(bhi, 0), shift(bhi, -1))
        t2 = seg_blocks(shift(bhi, 1), shift(blo, 1))
        t3 = seg_blocks(shift(blo, 0), shift(blo, -1))
        ones = np.ones((128, NSEG, RC), np.float32)
        xall = np.concatenate([t1, t2, t3, ones], axis=2)

        g0c = g0f[b, :, 16 * half:16 * half + 16, :].reshape(64, NPX)
        cstc = np.zeros((128, 2 + HPX), np.float32)
        cstc[0:64, 0] = 1.0 / 64.0
        cstc[64:128, 0] = 1.0
        cstc[0:64, 1] = 0.0
        cstc[64:128, 1] = -0.5
        cstc[0:64, 2:] = g0c[:, 0:HPX]
        cstc[64:128, 2:] = g0c[:, HPX:NPX]
        in_maps.append({
            "xs": xall.reshape(128, -1).astype(F8),
            "wt": wtt,
            "cst": cstc,
        })
    return in_maps


def _unpack(results):
    outf = np.empty((B, T, C, 32, 32), np.float32)
    for c in range(8):
        b, half = c // 2, c % 2
        arr = np.asarray(results[c]["out"], dtype=np.float32)  # [NSEG,128,NF2]
        hs = arr.reshape(NSEG, 2, 64, HPX, TS)[:, :, :, :, 1:]
        hs = hs.transpose(0, 4, 2, 1, 3).reshape(T, C, NPX)
        outf[b, :, :, 16 * half:16 * half + 16, :] = hs.reshape(T, C, 16, 32)
    return outf.reshape(B * T, C, 32, 32)


def kernel(x, conv_w, conv_b, h0):
    if "nc" not in _CACHE:
        _CACHE["nc"] = _build()
    nc = _CACHE["nc"]
    in_maps = _prep(x, conv_w, conv_b, h0)
    _CACHE["in_maps"] = in_maps
    res = run_bass_kernel_spmd(nc, in_maps, core_ids=list(range(8)))
    return _unpack(res.results)
